# revision 1
# baseline (speedup 1.0000x reference)
"""NT-Xent loss kernel for Trainium2, 8 NeuronCores.

Problem: B=4096 per view, D=128, temperature=0.1.
reps = concat([zjs, zis]) -> [8192, 128]; normalize rows; sim = normed @ normed.T;
loss = mean_i(-pos_i/T + logsumexp_{j!=i}(sim_ij/T)).

Strategy (fully static SPMD, no collectives) — exploits sim symmetry to
halve the exp work vs a full row-block scan:
  The 8192 rows form 64 tiles of 128.  Row tile T computes only the
  column band [T, T+32] (33 tiles, contiguous in the per-core rotated
  frame): the diagonal tile contributes row sums only; tiles T+1..T+31
  contribute row sums AND column sums (the transposed half of each
  pair); tile T+32 contributes both at host weight 0.5 (pairs at tile
  distance 32 are computed from both sides).  Every unordered pair then
  lands in r_i / r_j exactly once, so the exp covers the full matrix at
  half the cost.  Per core: its 8 row tiles (1024 rows); the rotated
  input keeps the band contiguous, so only 44 of 64 column tiles are
  touched.

  Host prep (like the rotation/tiling/bf16 packing, O(N*D) work that is
  0.01% of the O(N^2*D) kernel): normalize rows in f32 and ship the
  bf16 matmul operand directly in transposed [128d x rows] layout, plus
  a small row-major copy of the 16 tiles needed for pos/diag dots.

  Device, per (row tile, strip in {1536,1536,1152} of the 4224 band):
    PE    sim matmuls -> PSUM  (stationary = row tile, moving = band)
    ACT   exp(10x-10) PSUM -> SBUF bf16 E   (pure exp stream — the
          bottleneck engine runs back-to-back activations)
    DVE   tensor_scalar accumulate row sums of E (+ separate tail sum)
    PE    per-128-col-tile matmul, E as stationary and a ones column as
          moving -> column sums [128,1] into a persistent PSUM
          accumulator (partition-dense, cheap to drain)
  pos_i / diag_i via DVE dot products; diag uses the same bf16 values
  the matmul sees, so the host's exp(10*diag-10) subtraction removes
  the self column exactly.  A dozen warm-up matmuls run during the load
  phase so the PE p-state ramps before the strip pipeline starts.
  Host combines row/col partials (0.5 weight on the distance-32 tail),
  subtracts exp(10 diag - 10), takes log and averages in f64.
"""

import numpy as np

B = 4096
D = 128
TWO_B = 2 * B
P = 128
NCORES = 8
ROWS_PER_CORE = TWO_B // NCORES  # 1024
MI = 8                    # row tiles per core (128 rows each)
NTILES_IN = 44            # band cols reach local tile 40; pad to 44
NSLICES = NTILES_IN // 4
STRIPS = ((0, 1536), (1536, 1536), (3072, 1152))
INV_T = 10.0              # 1 / temperature
SHIFT = 10.0              # fixed logsumexp shift (sim/T <= 10)
OUT_W = 48 + MI * 32      # rowsum/tail/pos/diag block + colsum block

_CACHE = {}


def build_nc():
    import concourse.bacc as bacc
    import concourse.bass as bass
    import concourse.mybir as mybir
    import concourse.tile as tile

    f32 = mybir.dt.float32
    bf16 = mybir.dt.bfloat16
    OP = mybir.AluOpType
    AF = mybir.ActivationFunctionType

    # Pin the act-table chooser to the one set that holds Exp so no
    # mid-kernel ACT_TABLE_LOADs are emitted.
    from concourse import hw_specs

    _orig_tables = hw_specs.get_activation_tables

    def _patched_tables(arch):
        t = {k: set(v) for k, v in _orig_tables(arch).items()}
        for name, s in t.items():
            if name != "natural_log_exp_and_others":
                s.discard(AF.Exp)
                s.discard(AF.Ln)
        return t

    bacc.get_activation_tables = _patched_tables

    nc = bacc.Bacc(
        "TRN2",
        target_bir_lowering=False,
        debug=False,
        num_devices=NCORES,
    )
    # hit[d, 128t+p] = bf16(normed_rot[128t+p, d])  (transposed layout)
    hit_h = nc.declare_dram_parameter("hit", [P, NTILES_IN * P], bf16,
                                      isOutput=False)
    # hirows[p, k*128+d] = bf16(normed_rot[128*T_k+p, d]), T_k: 0..7,32..39
    hir_h = nc.declare_dram_parameter("hirows", [P, 16 * P], bf16,
                                      isOutput=False)
    out_h = nc.declare_dram_parameter("out", [P, OUT_W], f32, isOutput=True)

    with tile.TileContext(nc) as tc:
        with (
            tc.tile_pool(name="persist", bufs=1) as persist,
            tc.tile_pool(name="psum", bufs=2, space="PSUM") as psum,
            tc.tile_pool(name="psumacc", bufs=1, space="PSUM") as psumacc,
            tc.tile_pool(name="escr", bufs=4) as escr,
        ):
            HIT = persist.tile([P, NTILES_IN * P], bf16)
            HIR = persist.tile([P, 16, P], bf16)
            OUTBUF = persist.tile([P, 48], f32)
            ONES = persist.tile([P, 1], bf16)
            JP = persist.tile([P, P], bf16)
            JB = persist.tile([P, 1536], bf16)
            JT = persist.tile([P, P], bf16)
            bias_shift = persist.tile([P, 1], f32)
            nc.vector.memset(ONES, 1.0)
            nc.vector.memset(bias_shift, -SHIFT)

            COLACC = psumacc.tile([P, MI, 32], f32)
            PREHEAT = psumacc.tile([P, 512], f32)
            CSOUT = persist.tile([P, MI * 32], f32)

            # ---------------- loads + PE warm-up ----------------------------
            for s in range(NSLICES):
                x, y = 4 * s * P, (4 * s + 4) * P
                nc.gpsimd.dma_start(out=HIT[:, x:y], in_=hit_h[:, x:y])
            nc.scalar.dma_start(
                out=HIR,
                in_=hir_h[:, :].rearrange("p (k d) -> p k d", d=P),
            )
            for _ in range(3):
                nc.tensor.matmul(
                    PREHEAT, HIT[:, 0:P], HIT[:, 0:512],
                    start=True, stop=True,
                )

            def dots(step):
                # diag_i = ||h_i||^2 exactly as the matmul computes it (same
                # bf16 inputs, fp32 accumulation); pos_i = h_i . h_{i+4096}
                for mi in range(MI):
                    k2 = mi if step == 0 else 8 + mi
                    slot = 40 + mi if step == 0 else 32 + mi
                    nc.vector.scalar_tensor_tensor(
                        out=JP, in0=HIR[:, mi, :], scalar=1.0,
                        in1=HIR[:, k2, :], op0=OP.mult, op1=OP.mult,
                        accum_out=OUTBUF[:, slot : slot + 1],
                    )

            # ---------------- strips: sim + exp + row/col sums --------------
            # Strip-major (all A, then B, then C) so early strips only need
            # early HIT slices.  Colsum matmuls queue one strip behind the
            # sims; row sums all on DVE so ACT is a pure exp stream.
            pending_cs = []

            def flush_colsums():
                for lhs, t, m in pending_cs:
                    nc.tensor.matmul(
                        COLACC[:, t, m : m + 1], lhs, ONES,
                        start=True, stop=True,
                    )
                pending_cs.clear()

            for si, (off, w) in enumerate(STRIPS):
                for t in range(MI):
                    base = P * t
                    pg = psum.tile([P, 1536], f32, tag="pg")
                    for k in range(0, w, 512):
                        kw = min(512, w - k)
                        nc.tensor.matmul(
                            pg[:, k : k + kw],
                            HIT[:, base : base + P],
                            HIT[:, base + off + k : base + off + k + kw],
                            start=True, stop=True,
                        )
                    flush_colsums()
                    if si == 2 and t == 7:
                        # colacc rows for t<=6 are complete: drain them now
                        # so only t=7's 32 columns remain after the last exp
                        nc.vector.tensor_scalar(
                            out=CSOUT[:, 0:224],
                            in0=COLACC.rearrange("p t m -> p (t m)")[:, 0:224],
                            scalar1=1.0, scalar2=None, op0=OP.mult,
                        )
                        nc.sync.dma_start(out=out_h[:, 48 : 48 + 224],
                                          in_=CSOUT[:, 0:224])
                    E = escr.tile([P, 1536], bf16, tag="e")
                    if si == 2:
                        # C sweep: rowsum rides the ACT accumulator (DVE
                        # handles the tail), A/B sweeps use DVE
                        nc.scalar.activation(
                            out=E[:, :w], in_=pg[:, :w], func=AF.Exp,
                            scale=INV_T, bias=bias_shift,
                            accum_out=OUTBUF[:, 3 * t + si : 3 * t + si + 1],
                        )
                        nc.vector.tensor_scalar(
                            out=JT, in0=E[:, 1024:1152], scalar1=1.0,
                            scalar2=0.0, op0=OP.mult, op1=OP.add,
                            accum_out=OUTBUF[:, 24 + t : 25 + t],
                        )
                    else:
                        nc.scalar.activation(
                            out=E[:, :w], in_=pg[:, :w], func=AF.Exp,
                            scale=INV_T, bias=bias_shift,
                        )
                        nc.vector.tensor_scalar(
                            out=JB[:, :w], in0=E[:, :w], scalar1=1.0,
                            scalar2=0.0, op0=OP.mult, op1=OP.add,
                            accum_out=OUTBUF[:, 3 * t + si : 3 * t + si + 1],
                        )
                    # column sums: E tile stationary, ones moving -> [128,1]
                    j0 = 1 if si == 0 else 0
                    mbase = (0, 11, 23)[si]
                    for j in range(j0, w // P):
                        pending_cs.append(
                            (E[:, j * P : (j + 1) * P], t, mbase + j - j0)
                        )
                    if si == 0 and t == 1:
                        dots(0)
                    elif si == 0 and t == 2:
                        dots(1)
            flush_colsums()

            nc.vector.tensor_scalar(
                out=CSOUT[:, 224:256],
                in0=COLACC.rearrange("p t m -> p (t m)")[:, 224:256],
                scalar1=1.0, scalar2=None, op0=OP.mult,
            )
            nc.sync.dma_start(out=out_h[:, 0:48], in_=OUTBUF)
            nc.sync.dma_start(out=out_h[:, 48 + 224 : OUT_W],
                              in_=CSOUT[:, 224:256])

    nc.compile()
    return nc


def get_nc():
    if "nc" not in _CACHE:
        _CACHE["nc"] = build_nc()
    return _CACHE["nc"]


def make_in_maps(zis: np.ndarray, zjs: np.ndarray):
    import ml_dtypes

    # representations in reference order: [zjs; zis], normalized rows
    # (f32 norms with the torch CosineSimilarity 1e-8 clamp)
    reps = np.concatenate(
        [np.asarray(zjs, np.float32), np.asarray(zis, np.float32)], axis=0
    )
    normed = (
        reps / np.maximum(np.linalg.norm(reps, axis=1, keepdims=True), 1e-8)
    ).astype(ml_dtypes.bfloat16)
    maps = []
    kt = np.r_[0:8, 32:40]
    for c in range(NCORES):
        rot = np.roll(normed, -ROWS_PER_CORE * c, axis=0)[: NTILES_IN * P]
        hit = np.ascontiguousarray(rot.T)            # [128 d, 5632 rows]
        hir = np.ascontiguousarray(
            rot.reshape(NTILES_IN, P, D)[kt].transpose(1, 0, 2).reshape(
                P, 16 * P
            )
        )
        maps.append({"hit": hit, "hirows": hir})
    return maps


def kernel(zis: np.ndarray, zjs: np.ndarray) -> np.ndarray:
    from concourse.bass_utils import run_bass_kernel_spmd

    nc = get_nc()
    in_maps = make_in_maps(zis, zjs)
    res = None
    for attempt in range(3):
        try:
            res = run_bass_kernel_spmd(nc, in_maps, core_ids=list(range(NCORES)))
            break
        except Exception:
            # transient device-unrecoverable states heal on re-execution
            if attempt == 2:
                raise
            import time as _time

            _time.sleep(5.0)

    # ---- host combine (f64) -------------------------------------------
    r = np.zeros(TWO_B, dtype=np.float64)
    pos = np.zeros(TWO_B, dtype=np.float64)
    diag = np.zeros(TWO_B, dtype=np.float64)

    p_idx = np.arange(P)
    t_idx = np.arange(MI)
    m_idx = np.arange(32)
    row_l = 128 * t_idx[None, :] + p_idx[:, None]              # [P, MI]
    col_l = (128 * (t_idx[None, :, None] + 1 + m_idx[None, None, :])
             + p_idx[:, None, None])                           # [P, MI, 32]
    cw = np.where(m_idx == 31, 0.5, 1.0)[None, None, :]

    for c, rr in enumerate(res.results):
        o = rr["out"].astype(np.float64)                       # [P, OUT_W]
        rsum = o[:, 0:24].reshape(P, MI, 3)
        tail = o[:, 24:32]
        csum = o[:, 48:OUT_W].reshape(P, MI, 32)
        g_row = (1024 * c + row_l) % TWO_B
        g_col = (1024 * c + col_l) % TWO_B
        np.add.at(r, g_row,
                  rsum[:, :, 0] + rsum[:, :, 1] + rsum[:, :, 2]
                  - 0.5 * tail)
        np.add.at(r, g_col, cw * csum)
        pos[g_row] = o[:, 32:40]
        diag[g_row] = o[:, 40:48]

    lse = np.log(r - np.exp(INV_T * diag - SHIFT)) + SHIFT
    loss = np.mean(-INV_T * pos + lse)
    return np.array(loss, dtype=np.float32)

